# revision 3
# baseline (speedup 1.0000x reference)
"""KAN GLU expert — tensor-parallel TRN2 Bass kernel, 8 NeuronCores.

Sharding (per the GLU-FFN hint): core d owns d_ff rows [512d, 512d+512) of
w1/w2 (base+spline) and the matching 512-column K-shard of w3; x replicated.
Each core computes h[:, Fd] for ALL 4096 tokens, then a partial L3 output;
partials are ReduceScattered per 512-token chunk (dm-block d to core d) and
reassembled on host. Input bytes/core drop 8x vs data-parallel (252MB→34MB).

Math identical to the data-parallel baseline: kan_linear = silu(x) @ bw.T +
spline via 8 shifted cardinal cubic B-spline slabs, fused into one PSUM
accumulation (f32r base slab + bf16 spline slabs).
"""
import numpy as np
import ml_dtypes

import concourse.bacc as bacc
import concourse.mybir as mybir
import concourse.tile as tile
from concourse.bass_utils import run_bass_kernel_spmd

F32 = mybir.dt.float32
F32R = mybir.dt.float32r
BF16 = mybir.dt.bfloat16
AF = mybir.ActivationFunctionType
ALU = mybir.AluOpType

DM = 1024           # d_model
DF = 4096           # d_ff
C = 8               # spline coefficients per input
NCORES = 8
NTOK = 4096         # total tokens (all processed by every core)
TCH = 512           # tokens per chunk
NCH = NTOK // TCH   # 8 chunks
FFS = DF // NCORES  # 512 d_ff rows owned per core
KI1 = DM // 128     # 8 k-tiles per slab, layers 1/2
KI3 = FFS // 128    # 4 k-tiles per slab, layer 3 (sharded K)

SQ_A = float(6.0 ** -0.5)
SQ_B = float((2.0 / 3.0) ** 0.5)

_BF16 = ml_dtypes.bfloat16


def _register_const(nc, value, dtype=F32):
    key = (dtype, float(value))
    if key in nc.const_aps.aps:
        return
    t = nc.alloc_sbuf_tensor(f"const-{dtype.name}-{value}", [128, 1], dtype)
    nc.gpsimd.memset(t.ap(), float(value))
    nc.const_aps.aps[key] = t.ap()


def _basis_strips(nc, wst, wsb, strips, c):
    """B_c slab for the given [(out_ap, x_ap, shape)] strips.

    t is bf16 (safe: t-rounding only matters where t<2, i.e. B!=0, where
    bf16 ULP<=0.008). Both strips' t ops are emitted first so the in-order
    Act queue never blocks strip 1's head behind strip 0's squares. All DVE
    ops are plain tensor-tensor/tensor-scalar in the 2x (2-byte) mode —
    scalar_tensor_tensor runs at 1x and is avoided. Signs fold so the final
    combine is a subtract (on GPSIMD/Pool):
      a = min(t-2,0) = -r;  b = min(a+1,0) = -q
      u' = (a/sqrt6)^2 * a = -r^3/6;  v' = (sqrt(2/3)b)^2 * b = -(2/3)q^3
      B = v' - u' = r^3/6 - (2/3) q^3
    """
    ts = _basis_t(nc, wst, strips, c)
    _basis_rest(nc, wsb, strips, c, ts)


def _basis_t(nc, wst, strips, c):
    ts = []
    for (out_ap, x_ap, shape) in strips:
        t = wst.tile(shape, BF16, tag="wst", name=f"t_{c}")
        nc.scalar.activation(t[:], x_ap, AF.Abs, bias=float(3.5 - c), scale=2.5)
        ts.append(t)
    return ts


def _basis_rest(nc, wsb, strips, c, ts):
    for (out_ap, x_ap, shape), t in zip(strips, ts):
        a = wsb.tile(shape, BF16, tag="wsb", name=f"a_{c}")
        nc.vector.tensor_scalar(a[:], t[:], 2.0, 0.0, ALU.subtract, ALU.min)
        b = wsb.tile(shape, BF16, tag="wsb", name=f"b_{c}")
        nc.vector.tensor_scalar(b[:], a[:], -1.0, 0.0, ALU.subtract, ALU.min)
        a2 = wsb.tile(shape, BF16, tag="wsb", name=f"a2_{c}")
        nc.scalar.activation(a2[:], a[:], AF.Square, scale=SQ_A)           # r^2/6
        b2 = wsb.tile(shape, BF16, tag="wsb", name=f"b2_{c}")
        nc.scalar.activation(b2[:], b[:], AF.Square, scale=SQ_B)           # (2/3)q^2
        u = wsb.tile(shape, BF16, tag="wsb", name=f"u_{c}")
        nc.vector.tensor_mul(u[:], a2[:], a[:])                            # -r^3/6
        v = wsb.tile(shape, BF16, tag="wsb", name=f"v_{c}")
        nc.vector.tensor_mul(v[:], b2[:], b[:])                            # -(2/3)q^3
        nc.gpsimd.tensor_tensor(out_ap, v[:], u[:], ALU.subtract)          # B_c


def build_program(repeat=1):
    nc = bacc.Bacc("TRN2", target_bir_lowering=False, debug=False, num_devices=NCORES)

    xs_d = nc.dram_tensor("xs", (128, KI1, NTOK), F32, kind="ExternalInput")
    wb12_d = nc.dram_tensor("wb12", (128, KI1, 1024), F32R, kind="ExternalInput")
    ws12_d = nc.dram_tensor("ws12", (C, 128, KI1, 1024), BF16, kind="ExternalInput")
    wb3_d = nc.dram_tensor("wb3", (128, KI3, 1024), F32R, kind="ExternalInput")
    ws3_d = nc.dram_tensor("ws3", (C, 128, KI3, 1024), BF16, kind="ExternalInput")
    out_d = nc.dram_tensor("out", (128, NCH, TCH), F32, kind="ExternalOutput")

    for c in range(C):
        _register_const(nc, 3.5 - c)
    nc.all_engine_barrier()

    with tile.TileContext(nc) as tc:
      for _rep in range(repeat):   # >1 only for exec-time measurement
        with tc.tile_pool(name="dram", bufs=1, space="DRAM") as dram:
            hbuf = dram.tile([128, KI3, NTOK], F32, name="hbuf")       # 8.4MB
            pbuf = dram.tile([NCH, 8, 128, TCH], F32, name="pbuf")     # 16.8MB
            rsout = dram.tile([NCH, 128, TCH], F32, name="rsout")      # 2.1MB

            # ---------------- layers 1+2 (GLU halves), tokens chunked ----------------
            with (
                tc.tile_pool(name="wbase", bufs=1) as wbasep,          # 32KB
                tc.tile_pool(name="wspl", bufs=3) as wsplp,            # 24KB
                tc.tile_pool(name="xld", bufs=2) as xldp,              # 32KB
                tc.tile_pool(name="silu", bufs=2) as silup,            # 32KB
                tc.tile_pool(name="Bsl", bufs=3) as Bslp,              # 24KB
                tc.tile_pool(name="tws", bufs=2) as twsp,              # 8KB
                tc.tile_pool(name="bws", bufs=10) as bwsp,             # 40KB
                tc.tile_pool(name="hst", bufs=1) as hstp,              # 8KB
                tc.tile_pool(name="glt", bufs=2) as gltp,              # 4KB
                tc.tile_pool(name="ps12", bufs=1, space="PSUM") as ps12,
            ):
                wbase = wbasep.tile([128, KI1, 1024], F32R, name="wbase")
                for q in range(4):   # split so the first base matmuls start early
                    nc.sync.dma_start(wbase[:, 2 * q:2 * q + 2, :],
                                      wb12_d[:, 2 * q:2 * q + 2, :])

                # software-pipelined emission: x/silu prefetched one chunk
                # ahead, basis slabs computed one c ahead of their matmuls,
                # so PE never waits on the Act->DVE->Pool slab chain.
                xts, sils, Bts = {}, {}, {}

                def load_x(n):
                    xt = xldp.tile([128, KI1, TCH], F32, tag="x", name=f"x_{n}")
                    nc.sync.dma_start(xt[:], xs_d[:, :, n * TCH:(n + 1) * TCH])
                    sil = silup.tile([128, KI1, TCH], F32R, tag="sil", name=f"sil_{n}")
                    nc.scalar.activation(sil[:], xt[:], AF.Silu)
                    xts[n], sils[n] = xt, sil

                pend = {}

                def _strips_A(n, c):
                    Bt = Bslp.tile([128, KI1, TCH], BF16, tag="B", name=f"B_{n}_{c}")
                    Bts[(n, c)] = Bt
                    return [(Bt[:, 4 * s:4 * s + 4, :],
                             xts[n][:, 4 * s:4 * s + 4, :],
                             [128, 4, TCH]) for s in range(2)]

                def basis_A(n, c):
                    _basis_strips(nc, twsp, bwsp, _strips_A(n, c), c)

                def basis_A_t(n, c):
                    strips = _strips_A(n, c)
                    pend[(n, c)] = (strips, _basis_t(nc, twsp, strips, c))

                def basis_A_rest(n, c):
                    strips, ts = pend.pop((n, c))
                    _basis_rest(nc, bwsp, strips, c, ts)

                load_x(0)
                basis_A(0, 0)
                basis_A(0, 1)
                for n in range(NCH):
                    acc = [ps12.tile([128, TCH], F32, tag=f"ps{m}", name=f"ps_{n}_{m}")
                           for m in range(8)]
                    # base: f32r silu slab x f32r base weights
                    for ki in range(KI1):
                        for m in range(8):
                            nc.tensor.matmul(
                                acc[m][:], wbase[:, ki, 128 * m:128 * (m + 1)],
                                sils[n][:, ki, :], start=(ki == 0), stop=False)
                    # spline: bf16 B slabs x bf16 spline weights (streamed per c)
                    # basis pipelined two c ahead of its matmuls
                    for c in range(C):
                        if c == 4 and n + 1 < NCH:
                            load_x(n + 1)
                        if c + 2 < C:
                            basis_A(n, c + 2)
                        elif c + 2 == C + 1 and n + 1 < NCH:
                            basis_A_t(n + 1, 0)
                        Bt = Bts.pop((n, c))
                        for half in range(2):
                            wst = wsplp.tile([128, 4, 1024], BF16, tag="ws",
                                             name=f"ws_{n}_{c}_{half}")
                            nc.sync.dma_start(
                                wst[:], ws12_d[c, :, 4 * half:4 * half + 4, :])
                            for r in range(4):
                                ki = 4 * half + r
                                last = (c == C - 1 and ki == KI1 - 1)
                                for m in range(8):
                                    nc.tensor.matmul(
                                        acc[m][:], wst[:, r, 128 * m:128 * (m + 1)],
                                        Bt[:, ki, :], start=False, stop=last)
                    # GLU: h rows Fd = L1 * L2 ; spill h chunk to HBM (f32)
                    ht = hstp.tile([128, KI3, TCH], F32, tag="h", name=f"h_{n}")
                    for t in range(KI3):
                        tmp = gltp.tile([128, TCH], F32, tag="gt", name=f"gt_{n}_{t}")
                        if t < 2:      # split over Act/DVE so banks free in parallel
                            nc.scalar.copy(tmp[:], acc[t][:])
                        else:
                            nc.vector.tensor_copy(tmp[:], acc[t][:])
                        nc.vector.tensor_mul(ht[:, t, :], tmp[:], acc[4 + t][:])
                        if t == 1:
                            nc.sync.dma_start(
                                hbuf[:, 0:2, n * TCH:(n + 1) * TCH], ht[:, 0:2, :])
                    nc.sync.dma_start(hbuf[:, 2:4, n * TCH:(n + 1) * TCH], ht[:, 2:4, :])
                    if n + 1 < NCH:
                        basis_A_rest(n + 1, 0)
                        basis_A(n + 1, 1)
                    del xts[n], sils[n]

            # ---------------- layer 3 (partial over K-shard) + chunked RS ----------------
            with (
                tc.tile_pool(name="w3b", bufs=1) as w3bp,              # 16KB
                tc.tile_pool(name="w3s", bufs=1) as w3sp,              # 64KB
                tc.tile_pool(name="hld", bufs=2) as hldp,              # 16KB
                tc.tile_pool(name="sil3", bufs=2) as sil3p,            # 16KB
                tc.tile_pool(name="B3sl", bufs=9) as B3slp,            # 36KB
                tc.tile_pool(name="tws3", bufs=3) as tws3p,            # 6KB
                tc.tile_pool(name="bws3", bufs=16) as bws3p,           # 32KB
                tc.tile_pool(name="ost", bufs=1) as ostp,              # 16KB
                tc.tile_pool(name="ps3", bufs=1, space="PSUM") as ps3,
            ):
                w3b = w3bp.tile([128, KI3, 1024], F32R, name="w3b")
                for q in range(2):
                    nc.sync.dma_start(w3b[:, 2 * q:2 * q + 2, :],
                                      wb3_d[:, 2 * q:2 * q + 2, :])
                w3s = [w3sp.tile([128, KI3, 1024], BF16, tag=f"w3s{c}", name=f"w3s{c}")
                       for c in range(C)]

                # whole-chunk slab production: all 8 B3 slabs for chunk n are
                # emitted during chunk n-1's matmul window, so chunk n's
                # matmuls never wait on the Act->DVE->Pool slab chain.
                hts, sil3s, B3s = {}, {}, {}

                def load_h(n):
                    ht = hldp.tile([128, KI3, TCH], F32, tag="h", name=f"hl_{n}")
                    nc.sync.dma_start(ht[:], hbuf[:, :, n * TCH:(n + 1) * TCH])
                    sil3 = sil3p.tile([128, KI3, TCH], F32R, tag="s3", name=f"s3_{n}")
                    nc.scalar.activation(sil3[:], ht[:], AF.Silu)
                    hts[n], sil3s[n] = ht, sil3

                def basis_B(n, c):
                    B3 = B3slp.tile([128, KI3, TCH], BF16, tag="B3", name=f"B3_{n}_{c}")
                    _basis_strips(nc, tws3p, bws3p,
                                  [(B3[:, 2 * s:2 * s + 2, :],
                                    hts[n][:, 2 * s:2 * s + 2, :],
                                    [128, 2, TCH]) for s in range(2)], c)
                    B3s[(n, c)] = B3

                load_h(0)
                for c in range(C):   # interleave weight DMAs with prologue slabs
                    nc.sync.dma_start(w3s[c][:], ws3_d[c])
                    basis_B(0, c)
                for n in range(NCH):
                    acc3 = [ps3.tile([128, TCH], F32, tag=f"o{m}", name=f"ps3_{n}_{m}")
                            for m in range(8)]
                    for ki in range(KI3):
                        for m in range(8):
                            nc.tensor.matmul(
                                acc3[m][:], w3b[:, ki, 128 * m:128 * (m + 1)],
                                sil3s[n][:, ki, :], start=(ki == 0), stop=False)
                    for c in range(C):
                        if n + 1 < NCH:
                            if c == 0:
                                load_h(n + 1)
                            if c < 5:   # slabs 5..7 are emitted after the drains
                                basis_B(n + 1, c)
                        B3 = B3s.pop((n, c))
                        for ki in range(KI3):
                            last = (c == C - 1 and ki == KI3 - 1)
                            for m in range(8):
                                nc.tensor.matmul(
                                    acc3[m][:], w3s[c][:, ki, 128 * m:128 * (m + 1)],
                                    B3[:, ki, :], start=False, stop=last)
                    # drain to SBUF, stage to HBM partial (dm-block major), RS chunk
                    ost = ostp.tile([128, 8, TCH], F32, tag="ost", name=f"ost_{n}")
                    for m in range(8):
                        if m % 2 == 0:   # split over Act/DVE: banks free in parallel
                            nc.scalar.copy(ost[:, m, :], acc3[m][:])
                        else:
                            nc.vector.tensor_copy(ost[:, m, :], acc3[m][:])
                    for m in range(8):
                        nc.sync.dma_start(pbuf[n, m], ost[:, m, :])
                    nc.gpsimd.collective_compute(
                        "ReduceScatter",
                        ALU.add,
                        replica_groups=[list(range(NCORES))],
                        ins=[pbuf[n].opt()],
                        outs=[rsout[n].opt()],
                    )
                    nc.sync.dma_start(out_d[:, n, :], rsout[n])
                    if n + 1 < NCH:
                        for cc in (5, 6, 7):
                            basis_B(n + 1, cc)
                    del hts[n], sil3s[n]

    nc.compile()
    return nc


def pack_weights(base_w1, spline_w1, base_w2, spline_w2, base_w3, spline_w3):
    """Per-core weight shards, matmul-ready layouts."""
    f32 = np.float32
    bw1 = np.asarray(base_w1, f32)
    bw2 = np.asarray(base_w2, f32)
    sw1 = np.asarray(spline_w1, f32)
    sw2 = np.asarray(spline_w2, f32)
    bw3 = np.asarray(base_w3, f32)
    sw3 = np.asarray(spline_w3, f32)
    shards = []
    for d in range(NCORES):
        sl = slice(FFS * d, FFS * (d + 1))
        W = np.concatenate([bw1[sl], bw2[sl]], axis=0)          # (1024 M, 1024 K)
        wb12 = np.ascontiguousarray(
            W.T.reshape(KI1, 128, 1024).transpose(1, 0, 2))      # (128, 8, 1024)
        S = np.concatenate([sw1[sl], sw2[sl]], axis=0)           # (1024 M, 1024 K, C)
        ws12 = np.ascontiguousarray(
            S.transpose(2, 1, 0).reshape(C, KI1, 128, 1024)
            .transpose(0, 2, 1, 3)).astype(_BF16)                # (C, 128, 8, 1024)
        wb3 = np.ascontiguousarray(
            bw3[:, sl].T.reshape(KI3, 128, 1024).transpose(1, 0, 2))  # (128, 4, 1024)
        ws3 = np.ascontiguousarray(
            sw3[:, sl, :].transpose(2, 1, 0).reshape(C, KI3, 128, 1024)
            .transpose(0, 2, 1, 3)).astype(_BF16)                # (C, 128, 4, 1024)
        shards.append((wb12, ws12, wb3, ws3))
    return shards


_prog_cache = {}


def kernel(x, base_w1, spline_w1, base_w2, spline_w2, base_w3, spline_w3,
           grid_in=None, grid_ff=None):
    x = np.asarray(x, np.float32)
    shp = x.shape
    x2 = x.reshape(-1, DM)                       # (4096, 1024)
    assert x2.shape[0] == NTOK

    xs = np.ascontiguousarray(
        x2.T.reshape(KI1, 128, NTOK).transpose(1, 0, 2))         # (128, 8, 4096)
    shards = pack_weights(base_w1, spline_w1, base_w2, spline_w2,
                          base_w3, spline_w3)

    if "nc" not in _prog_cache:
        _prog_cache["nc"] = build_program()
    nc = _prog_cache["nc"]

    in_maps = [{"xs": xs, "wb12": sh[0], "ws12": sh[1], "wb3": sh[2], "ws3": sh[3]}
               for sh in shards]
    res = run_bass_kernel_spmd(nc, in_maps, core_ids=list(range(NCORES)))

    out = np.empty((NTOK, DM), np.float32)
    for d in range(NCORES):
        o = res.results[d]["out"]                # (128, 8, 512) = [p, n, t]
        out[:, 128 * d:128 * (d + 1)] = o.transpose(1, 2, 0).reshape(NTOK, 128)
    return out.reshape(shp)
